# revision 5
# baseline (speedup 1.0000x reference)
"""CLoRALinear Trainium2 kernel (fp8-hybrid v2).

Computes y = x @ (W + (alpha/r) * A @ B.T).T + bias for
x:[4,2048,4096] f32, W:[4096,4096], bias:[4096], A:[4096,32], B:[4096,32].

Strategy: data-parallel over tokens across 8 NeuronCores (1024 tokens each).
Per core the contraction dim (4096 = 32 k-tiles of 128) is split:
  k-tiles  0..KTB-1  : bf16 matmuls (fp32 PSUM accum)
  k-tiles KTB..31    : fp8e4 DoubleRow matmuls (2 k-tiles per instruction,
                       2x PE throughput)
The fp8 split fraction is chosen so the end-to-end rel err stays ~1.6e-2,
under the 2e-2 gate.  W (std 0.02) would be subnormal in e4m3, so the fp8
path carries a x256 scale: W.T tiles are produced by PE transposes whose
"identity" moving tensor is 256*I, making PSUM = 256*y; the y copy-out is
an ACT copy with scale 1/256.  B is likewise scaled x256 (so u = 256*x@B),
and A/bias enter unscaled/256-scaled respectively via the augmented LoRA
matmul [u ; 1] @ [A.T ; 256*bias].

x.T and W.T tiles are produced on-chip by PE transposes (fp32 inputs have no
DMA-transpose path; fp32->bf16 casts ride the SWDGE loads).  To keep the PE
HAM clock warm, W.T transposes for slice n+1 are interleaved after the m-tile
matmul groups of slice n instead of running as one long burst, and transpose
results are batched 4-8-per-PSUM-bank with a single copy out (alternating
DVE/ACT) so copies never gate the PE.
"""

import sys

sys.path.insert(0, "/opt/trn_rl_repo")

import numpy as np

import concourse.bass as bass
import concourse.tile as tile
from concourse import bacc, mybir
from concourse.bass_utils import run_bass_kernel_spmd
from concourse.masks import make_identity

F32 = mybir.dt.float32
BF16 = mybir.dt.bfloat16
FP8 = mybir.dt.float8e4
DR = mybir.MatmulPerfMode.DoubleRow

N_CORES = 8
TOK = 1024          # tokens per core
DIN = 4096
DOUT = 4096
R = 32
KT = DIN // 128     # 32 k-tiles
KT8 = 8             # fp8 k-tiles (last KT8 of KT; must be even)
KTB = KT - KT8      # bf16 k-tiles
MT = TOK // 128     # 8 m-tiles
NSL = 512           # out-features per n-slice
NT = DOUT // NSL    # 8 n-slices
CPS = NSL // 128    # 4 weight chunks per n-slice
SW = 256.0          # fp8/W scale (power of two; PSUM holds 256*y)

_cached = None


def _build():
    nc = bacc.Bacc("TRN2", target_bir_lowering=False, debug=False)

    x_d = nc.dram_tensor("x", [TOK, DIN], F32, kind="ExternalInput").ap()
    w_d = nc.dram_tensor("weight", [DOUT, DIN], F32, kind="ExternalInput").ap()
    bias_d = nc.dram_tensor("bias", [DOUT], F32, kind="ExternalInput").ap()
    a_d = nc.dram_tensor("A", [DOUT, R], F32, kind="ExternalInput").ap()
    b_d = nc.dram_tensor("B", [DIN, R], F32, kind="ExternalInput").ap()
    y_d = nc.dram_tensor("out", [TOK, DOUT], F32, kind="ExternalOutput").ap()

    with tile.TileContext(nc) as tc:
        with (
            tc.tile_pool(name="const", bufs=1) as const_pool,
            tc.tile_pool(name="xchunk", bufs=2) as xchunk_pool,
            tc.tile_pool(name="wchunk", bufs=3) as wchunk_pool,
            tc.tile_pool(name="wT", bufs=2) as wT_pool,
            tc.tile_pool(name="yout", bufs=3) as y_pool,
            tc.tile_pool(name="tpsum", bufs=6, space="PSUM") as tpsum_pool,
            tc.tile_pool(name="ypsum", bufs=2, space="PSUM") as ypsum_pool,
        ):
            ident = const_pool.tile([128, 128], BF16)
            make_identity(nc, ident[:])
            ident_f32 = const_pool.tile([128, 128], F32)

            copy_idx = [0]

            def tcopy(dst, src):
                if copy_idx[0] % 2 == 0:
                    nc.vector.tensor_copy(dst, src)
                else:
                    nc.scalar.copy(dst, src)
                copy_idx[0] += 1

            def tcopy_scaled(dst, src, scale):
                # W.T copy-outs carry the fp8 x256 scale (PE transposes are
                # pure permutations, so the scale must ride the copy)
                if copy_idx[0] % 2 == 0:
                    nc.vector.tensor_scalar_mul(dst, src, scale)
                else:
                    nc.scalar.mul(dst, src, scale)
                copy_idx[0] += 1

            # ---- x phase: build x.T resident + u_aug; also W.T slice 0 ----
            # x_t holds bf16 x.T for the bf16 k-range; x8_t holds fp8 x.T
            # for the fp8 k-range (plus bf16 duplicate only transiently via
            # PSUM).  u is computed bf16 over k<KTB and fp8-DR over the rest.
            x_t = const_pool.tile([128, KTB, TOK], BF16)
            x8_t = const_pool.tile([128, KT8, TOK], FP8)
            u_aug = const_pool.tile([R + 1, TOK], BF16)

            # chunk 0 DMA hoisted ahead of the second identity build so the
            # first transposes start as early as possible
            x_chunk0 = xchunk_pool.tile(
                [128, DIN], BF16, tag="xchunk", name="x_chunk0"
            )
            nc.gpsimd.dma_start(x_chunk0[:, 0:DIN // 2], x_d[0:128, 0:DIN // 2])
            nc.gpsimd.dma_start(x_chunk0[:, DIN // 2:], x_d[0:128, DIN // 2:])
            make_identity(nc, ident_f32[:])
            nc.gpsimd.memset(u_aug[R:R + 1, :], 1.0)

            w_t = [
                wT_pool.tile([128, KTB, NSL], BF16, tag="wt", name="wt0"),
                wT_pool.tile([128, KTB, NSL], BF16, tag="wt", name="wt1"),
            ]
            w8_t = [
                wT_pool.tile([128, KT8, NSL], FP8, tag="wt8", name="w8t0"),
                wT_pool.tile([128, KT8, NSL], FP8, tag="wt8", name="w8t1"),
            ]
            w_chunks = {}

            def load_w_chunk(n, c):
                ch = wchunk_pool.tile([128, DIN], BF16, tag="wchunk")
                nc.gpsimd.dma_start(
                    ch[:], w_d[n * NSL + c * 128:n * NSL + (c + 1) * 128, :]
                )
                w_chunks[(n, c)] = ch

            def w_transpose_run(n, c, k0, nk):
                """Transpose nk k-blocks (k indices k0..) of chunk c of slice
                n into w_t (bf16, k<KTB) / w8_t (fp8, k>=KTB), batching one
                full PSUM bank (8 bf16 blocks) per copy; the copy applies
                the x256 fp8 scale."""
                ch = w_chunks[(n, c)]
                f32 = ch.dtype == F32
                bs = 4 if f32 else 8
                idn = ident_f32 if f32 else ident
                col0 = c * 128
                for b0 in range(k0, k0 + nk, bs):
                    pt = tpsum_pool.tile(
                        [128, bs, 128], F32 if f32 else BF16, tag="t"
                    )
                    for j in range(bs):
                        nc.tensor.transpose(
                            pt[:, j, :],
                            ch[:, (b0 + j) * 128:(b0 + j + 1) * 128],
                            idn[:],
                        )
                    if b0 >= KTB:
                        dst = w8_t[n % 2][:, b0 - KTB:b0 - KTB + bs,
                                          col0:col0 + 128]
                    else:
                        dst = w_t[n % 2][:, b0:b0 + bs, col0:col0 + 128]
                    tcopy_scaled(dst, pt[:], SW)

            def x_transpose_run(m, chunk):
                """Transpose all KT k-blocks of x m-chunk: k<KTB into x_t
                (bf16), k>=KTB into x8_t (fp8, unscaled cast)."""
                f32 = chunk.dtype == F32
                bs = 4 if f32 else 8
                idn = ident_f32 if f32 else ident
                col0 = m * 128
                for b0 in range(0, KT, bs):
                    pt = tpsum_pool.tile(
                        [128, bs, 128], F32 if f32 else BF16, tag="t"
                    )
                    for j in range(bs):
                        nc.tensor.transpose(
                            pt[:, j, :],
                            chunk[:, (b0 + j) * 128:(b0 + j + 1) * 128],
                            idn[:],
                        )
                    if b0 >= KTB:
                        dst = x8_t[:, b0 - KTB:b0 - KTB + bs, col0:col0 + 128]
                    else:
                        dst = x_t[:, b0:b0 + bs, col0:col0 + 128]
                    tcopy(dst, pt[:])

            # x chunks: even m via gpsimd cast-DMA (bf16, two halves for
            # pipelining); odd m via the sync ring as fp32 (transposed at
            # fp32, copy-cast to bf16/fp8).  Splits the load across queues.
            for m in range(MT):
                if m == 0:
                    x_chunk = x_chunk0
                elif m % 2 == 0:
                    x_chunk = xchunk_pool.tile(
                        [128, DIN], BF16, tag="xchunk", name="x_chunk"
                    )
                    nc.gpsimd.dma_start(
                        x_chunk[:, 0:DIN // 2],
                        x_d[m * 128:(m + 1) * 128, 0:DIN // 2],
                    )
                    nc.gpsimd.dma_start(
                        x_chunk[:, DIN // 2:],
                        x_d[m * 128:(m + 1) * 128, DIN // 2:],
                    )
                else:
                    x_chunk = xchunk_pool.tile(
                        [128, DIN], F32, tag="xf32", name="x_chunk", bufs=1
                    )
                    nc.sync.dma_start(
                        x_chunk[:, 0:DIN // 2],
                        x_d[m * 128:(m + 1) * 128, 0:DIN // 2],
                    )
                    nc.sync.dma_start(
                        x_chunk[:, DIN // 2:],
                        x_d[m * 128:(m + 1) * 128, DIN // 2:],
                    )
                if 1 <= m <= CPS:
                    load_w_chunk(0, m - 1)
                x_transpose_run(m, x_chunk)
                if m >= 4:
                    # build W.T slice 0: chunk m-4, both halves
                    w_transpose_run(0, m - 4, 0, 16)
                    w_transpose_run(0, m - 4, 16, 16)

            # constants (deferred: consumers all run after the x phase):
            # B (one gather DMA, then x256 scale + fp8 cast for the fp8
            # k-range), A_aug = [A.T ; 256*bias]
            b_all = const_pool.tile([128, KT, R], BF16)
            nc.gpsimd.dma_start(
                b_all[:], b_d.rearrange("(k p) r -> p k r", p=128)
            )
            nc.vector.tensor_scalar_mul(b_all[:], b_all[:], SW)
            b8_all = const_pool.tile([128, KT8, R], FP8)
            nc.scalar.copy(b8_all[:], b_all[:, KTB:, :])
            a_nat = const_pool.tile([128, DOUT // 128, R], BF16)
            nc.gpsimd.dma_start(
                a_nat[:], a_d.rearrange("(o p) r -> p o r", p=128)
            )
            a_aug = const_pool.tile([R + 1, DOUT], BF16)
            nc.gpsimd.dma_start(a_aug[R:R + 1, :], bias_d[None, :])
            nc.vector.tensor_scalar_mul(
                a_aug[R:R + 1, :], a_aug[R:R + 1, :], SW
            )
            for o in range(DOUT // 128):
                pt = tpsum_pool.tile([R, 128], BF16, tag="t")
                nc.tensor.transpose(pt[:], a_nat[:, o, :], ident[:])
                nc.vector.tensor_copy(a_aug[0:R, o * 128:(o + 1) * 128], pt[:])

            # u = 256*(x @ B).T over full token range, batched N=512 matmuls
            # (PSUM tiles borrowed from the ypsum pool slots); bf16 k-range
            # plus fp8-DR k-range.
            for mc in range(2):
                up = ypsum_pool.tile([R, NSL], F32, tag="y", name="up")
                for k in range(KTB):
                    nc.tensor.matmul(
                        up[:],
                        b_all[:, k, :],
                        x_t[:, k, mc * NSL:(mc + 1) * NSL],
                        start=(k == 0),
                        stop=False,
                    )
                for kk in range(KT8 // 2):
                    nc.tensor.matmul(
                        up[:],
                        b8_all[:, 2 * kk:2 * kk + 2, :],
                        x8_t[:, 2 * kk:2 * kk + 2, mc * NSL:(mc + 1) * NSL],
                        start=False,
                        stop=(kk == KT8 // 2 - 1),
                        perf_mode=DR,
                        skip_group_check=True,
                    )
                tcopy(u_aug[0:R, mc * NSL:(mc + 1) * NSL], up[:])

            # ---- main loop over output-feature slices ----
            for n in range(NT):
                cur = w_t[n % 2]
                cur8 = w8_t[n % 2]
                for m in range(MT):
                    if n + 1 < NT:
                        # chunk c of slice n+1 is consumed at m=2c and 2c+1;
                        # load it one m-iteration ahead (c=0 at m=0).
                        if m == 0:
                            load_w_chunk(n + 1, 0)
                        if m % 2 == 1 and (m + 1) // 2 < CPS:
                            load_w_chunk(n + 1, (m + 1) // 2)
                    yp = ypsum_pool.tile([128, NSL], F32, tag="y")
                    for k in range(KTB):
                        nc.tensor.matmul(
                            yp[:],
                            x_t[:, k, m * 128:(m + 1) * 128],
                            cur[:, k, :],
                            start=(k == 0),
                            stop=False,
                        )
                    for kk in range(KT8 // 2):
                        nc.tensor.matmul(
                            yp[:],
                            x8_t[:, 2 * kk:2 * kk + 2, m * 128:(m + 1) * 128],
                            cur8[:, 2 * kk:2 * kk + 2, :],
                            start=False,
                            stop=False,
                            perf_mode=DR,
                            skip_group_check=True,
                        )
                    nc.tensor.matmul(
                        yp[:],
                        u_aug[:, m * 128:(m + 1) * 128],
                        a_aug[:, n * NSL:(n + 1) * NSL],
                        start=False,
                        stop=True,
                        skip_group_check=True,
                    )
                    y_sb = y_pool.tile([128, NSL], F32, tag="ysb")
                    nc.scalar.mul(y_sb[:], yp[:], 1.0 / SW)
                    nc.sync.dma_start(
                        y_d[m * 128:(m + 1) * 128, n * NSL:(n + 1) * NSL],
                        y_sb[:],
                    )
                    if n + 1 < NT:
                        # 16 transposes of slice n+1 after each m's matmuls
                        w_transpose_run(n + 1, m // 2, (m % 2) * 16, 16)

    nc.compile()
    return nc


def _get_nc():
    global _cached
    if _cached is None:
        _cached = _build()
    return _cached


def kernel(x, weight, bias, A, B, _trace=False):
    x = np.ascontiguousarray(np.asarray(x, dtype=np.float32)).reshape(-1, DIN)
    weight = np.ascontiguousarray(np.asarray(weight, dtype=np.float32))
    bias = np.ascontiguousarray(np.asarray(bias, dtype=np.float32))
    A = np.ascontiguousarray(np.asarray(A, dtype=np.float32))
    B = np.ascontiguousarray(np.asarray(B, dtype=np.float32))

    nc = _get_nc()
    in_maps = [
        {
            "x": np.ascontiguousarray(x[c * TOK:(c + 1) * TOK]),
            "weight": weight,
            "bias": bias,
            "A": A,
            "B": B,
        }
        for c in range(N_CORES)
    ]
    res = run_bass_kernel_spmd(
        nc, in_maps, core_ids=list(range(N_CORES)), trace=_trace
    )
    kernel.last_result = res
    y = np.concatenate([res.results[c]["out"] for c in range(N_CORES)], axis=0)
    return y.reshape(4, 2048, DOUT)


kernel.last_result = None


# revision 8
# speedup vs baseline: 1.0326x; 1.0326x over previous
"""CLoRALinear Trainium2 kernel (fp8-hybrid, pipelined startup).

Computes y = x @ (W + (alpha/r) * A @ B.T).T + bias for
x:[4,2048,4096] f32, W:[4096,4096], bias:[4096], A:[4096,32], B:[4096,32].

Strategy: data-parallel over tokens across 8 NeuronCores (1024 tokens each).
Per core the contraction dim (4096 = 32 k-tiles of 128) is split:
  k-tiles  0..KTB-1  : bf16 matmuls (fp32 PSUM accum)
  k-tiles KTB..31    : fp8e4 DoubleRow matmuls (2 k-tiles per instruction,
                       2x PE throughput; measured 216ns per DR instr = same
                       as one bf16 instr for twice the K)
The fp8 fraction (10/32) puts the end-to-end rel err at ~1.77e-2, under the
2e-2 gate.  W (std 0.02) would be subnormal in e4m3, so the fp8 path carries
a x256 scale applied by the W.T PSUM->SBUF copies (PE transposes are pure
permutations and cannot scale); PSUM = 256*y and the y copy-out is an ACT
copy with scale 1/256.  B is likewise scaled x256 (u = 256*x@B), and A/bias
enter via the augmented LoRA matmul [u ; 1] @ [A.T ; 256*bias].

x.T and W.T tiles are produced on-chip by PE transposes (fp32 inputs have no
DMA-transpose path; fp32->bf16 casts ride the SWDGE loads).  Startup is
pipelined: W slice 0 and x m-blocks 0..1 load+transpose first, then the n=0
main loop runs m-pairwise, interleaved with the remaining x transposes and
256-token u batches, so the PE never waits for the full x phase.  W.T
transposes for slice n+1 are interleaved after the m-tile matmul groups of
slice n, and transpose results are batched 4-8-per-PSUM-bank with a single
copy out (alternating DVE/ACT) so copies never gate the PE.
"""

import sys

sys.path.insert(0, "/opt/trn_rl_repo")

import numpy as np

import concourse.bass as bass
import concourse.tile as tile
from concourse import bacc, mybir
from concourse.bass_utils import run_bass_kernel_spmd
from concourse.masks import make_identity

F32 = mybir.dt.float32
BF16 = mybir.dt.bfloat16
FP8 = mybir.dt.float8e4
DR = mybir.MatmulPerfMode.DoubleRow

N_CORES = 8
TOK = 1024          # tokens per core
DIN = 4096
DOUT = 4096
R = 32
KT = DIN // 128     # 32 k-tiles
KT8 = 10            # fp8 k-tiles (last KT8 of KT; must be even)
KTB = KT - KT8      # bf16 k-tiles
MT = TOK // 128     # 8 m-tiles
NSL = 512           # out-features per n-slice
NT = DOUT // NSL    # 8 n-slices
CPS = NSL // 128    # 4 weight chunks per n-slice
SW = 256.0          # fp8/W scale (power of two; PSUM holds 256*y)

_cached = None


def _build():
    nc = bacc.Bacc("TRN2", target_bir_lowering=False, debug=False)

    x_d = nc.dram_tensor("x", [TOK, DIN], F32, kind="ExternalInput").ap()
    w_d = nc.dram_tensor("weight", [DOUT, DIN], F32, kind="ExternalInput").ap()
    bias_d = nc.dram_tensor("bias", [DOUT], F32, kind="ExternalInput").ap()
    a_d = nc.dram_tensor("A", [DOUT, R], F32, kind="ExternalInput").ap()
    b_d = nc.dram_tensor("B", [DIN, R], F32, kind="ExternalInput").ap()
    y_d = nc.dram_tensor("out", [TOK, DOUT], F32, kind="ExternalOutput").ap()

    with tile.TileContext(nc) as tc:
        with (
            tc.tile_pool(name="const", bufs=1) as const_pool,
            tc.tile_pool(name="xchunk", bufs=2) as xchunk_pool,
            tc.tile_pool(name="wchunk", bufs=4) as wchunk_pool,
            tc.tile_pool(name="wT", bufs=2) as wT_pool,
            tc.tile_pool(name="yout", bufs=3) as y_pool,
            tc.tile_pool(name="tpsum", bufs=6, space="PSUM") as tpsum_pool,
            tc.tile_pool(name="ypsum", bufs=2, space="PSUM") as ypsum_pool,
        ):
            ident = const_pool.tile([128, 128], BF16)
            make_identity(nc, ident[:])
            ident_f32 = const_pool.tile([128, 128], F32)

            copy_idx = [0]

            def tcopy(dst, src):
                if copy_idx[0] % 2 == 0:
                    nc.vector.tensor_copy(dst, src)
                else:
                    nc.scalar.copy(dst, src)
                copy_idx[0] += 1

            def tcopy_scaled(dst, src, scale):
                # W.T copy-outs carry the fp8 x256 scale (PE transposes are
                # pure permutations, so the scale must ride the copy)
                if copy_idx[0] % 2 == 0:
                    nc.vector.tensor_scalar_mul(dst, src, scale)
                else:
                    nc.scalar.mul(dst, src, scale)
                copy_idx[0] += 1

            x_t = const_pool.tile([128, KTB, TOK], BF16)
            x8_t = const_pool.tile([128, KT8, TOK], FP8)
            u_aug = const_pool.tile([R + 1, TOK], BF16)

            # ---- kick off startup DMAs ----
            x_chunks = {}

            def load_x_chunk(m):
                if m % 2 == 0:
                    ch = xchunk_pool.tile(
                        [128, DIN], BF16, tag="xchunk", name="x_chunk"
                    )
                    nc.gpsimd.dma_start(
                        ch[:, 0:DIN // 2], x_d[m * 128:(m + 1) * 128, 0:DIN // 2]
                    )
                    nc.gpsimd.dma_start(
                        ch[:, DIN // 2:], x_d[m * 128:(m + 1) * 128, DIN // 2:]
                    )
                else:
                    ch = xchunk_pool.tile(
                        [128, DIN], F32, tag="xf32", name="x_chunk", bufs=1
                    )
                    nc.sync.dma_start(
                        ch[:, 0:DIN // 2], x_d[m * 128:(m + 1) * 128, 0:DIN // 2]
                    )
                    nc.sync.dma_start(
                        ch[:, DIN // 2:], x_d[m * 128:(m + 1) * 128, DIN // 2:]
                    )
                x_chunks[m] = ch

            w_chunks = {}

            def load_w_chunk(n, c):
                ch = wchunk_pool.tile([128, DIN], BF16, tag="wchunk")
                nc.gpsimd.dma_start(
                    ch[:], w_d[n * NSL + c * 128:n * NSL + (c + 1) * 128, :]
                )
                w_chunks[(n, c)] = ch

            load_x_chunk(0)
            load_x_chunk(1)
            for c in range(CPS):
                load_w_chunk(0, c)
            b_all = const_pool.tile([128, KT, R], BF16)
            nc.gpsimd.dma_start(
                b_all[:], b_d.rearrange("(k p) r -> p k r", p=128)
            )
            a_nat = const_pool.tile([128, DOUT // 128, R], BF16)
            nc.gpsimd.dma_start(
                a_nat[:], a_d.rearrange("(o p) r -> p o r", p=128)
            )
            a_aug = const_pool.tile([R + 1, DOUT], BF16)
            nc.gpsimd.dma_start(a_aug[R:R + 1, :], bias_d[None, :])

            make_identity(nc, ident_f32[:])
            nc.gpsimd.memset(u_aug[R:R + 1, :], 1.0)

            w_t = [
                wT_pool.tile([128, KTB, NSL], BF16, tag="wt", name="wt0"),
                wT_pool.tile([128, KTB, NSL], BF16, tag="wt", name="wt1"),
            ]
            w8_t = [
                wT_pool.tile([128, KT8, NSL], FP8, tag="wt8", name="w8t0"),
                wT_pool.tile([128, KT8, NSL], FP8, tag="wt8", name="w8t1"),
            ]

            def split_ranges(b0, bs):
                """Split block range [b0, b0+bs) at the KTB dtype boundary."""
                if b0 >= KTB or b0 + bs <= KTB:
                    return [(b0, b0 + bs)]
                return [(b0, KTB), (KTB, b0 + bs)]

            def w_transpose_run(n, c, k0, nk):
                """Transpose nk k-blocks (k indices k0..) of chunk c of slice
                n into w_t (bf16, k<KTB) / w8_t (fp8, k>=KTB), batching one
                full PSUM bank per copy; copies apply the x256 fp8 scale."""
                ch = w_chunks[(n, c)]
                f32 = ch.dtype == F32
                bs = 4 if f32 else 8
                idn = ident_f32 if f32 else ident
                col0 = c * 128
                for b0 in range(k0, k0 + nk, bs):
                    pt = tpsum_pool.tile(
                        [128, bs, 128], F32 if f32 else BF16, tag="t"
                    )
                    for j in range(bs):
                        nc.tensor.transpose(
                            pt[:, j, :],
                            ch[:, (b0 + j) * 128:(b0 + j + 1) * 128],
                            idn[:],
                        )
                    for r0, r1 in split_ranges(b0, bs):
                        if r0 >= KTB:
                            dst = w8_t[n % 2][:, r0 - KTB:r1 - KTB,
                                              col0:col0 + 128]
                        else:
                            dst = w_t[n % 2][:, r0:r1, col0:col0 + 128]
                        tcopy_scaled(dst, pt[:, r0 - b0:r1 - b0, :], SW)

            def x_transpose_run(m):
                """Transpose all KT k-blocks of x m-chunk: k<KTB into x_t
                (bf16), k>=KTB into x8_t (fp8, unscaled cast)."""
                chunk = x_chunks.pop(m)
                f32 = chunk.dtype == F32
                bs = 4 if f32 else 8
                idn = ident_f32 if f32 else ident
                col0 = m * 128
                for b0 in range(0, KT, bs):
                    pt = tpsum_pool.tile(
                        [128, bs, 128], F32 if f32 else BF16, tag="t"
                    )
                    for j in range(bs):
                        nc.tensor.transpose(
                            pt[:, j, :],
                            chunk[:, (b0 + j) * 128:(b0 + j + 1) * 128],
                            idn[:],
                        )
                    for r0, r1 in split_ranges(b0, bs):
                        if r0 >= KTB:
                            dst = x8_t[:, r0 - KTB:r1 - KTB, col0:col0 + 128]
                        else:
                            dst = x_t[:, r0:r1, col0:col0 + 128]
                        tcopy(dst, pt[:, r0 - b0:r1 - b0, :])

            def u_batch(mc):
                """u[:, mc*256:(mc+1)*256] = 256*(x @ B).T for 2 m-tiles."""
                cols = slice(mc * 256, (mc + 1) * 256)
                up = ypsum_pool.tile([R, 256], F32, tag="y", name="up")
                for k in range(KTB):
                    nc.tensor.matmul(
                        up[:], b_all[:, k, :], x_t[:, k, cols],
                        start=(k == 0), stop=False,
                    )
                for kk in range(KT8 // 2):
                    nc.tensor.matmul(
                        up[:],
                        b8_all[:, 2 * kk:2 * kk + 2, :],
                        x8_t[:, 2 * kk:2 * kk + 2, cols],
                        start=False, stop=(kk == KT8 // 2 - 1),
                        perf_mode=DR, skip_group_check=True,
                    )
                tcopy(u_aug[0:R, cols], up[:])

            def main_tile(n, m):
                """One [128 tok, 512 out] output tile: bf16 k-loop + fp8-DR
                k-loop + augmented LoRA matmul, then scaled copy-out + DMA."""
                cur = w_t[n % 2]
                cur8 = w8_t[n % 2]
                yp = ypsum_pool.tile([128, NSL], F32, tag="y")
                for k in range(KTB):
                    nc.tensor.matmul(
                        yp[:], x_t[:, k, m * 128:(m + 1) * 128], cur[:, k, :],
                        start=(k == 0), stop=False,
                    )
                for kk in range(KT8 // 2):
                    nc.tensor.matmul(
                        yp[:],
                        x8_t[:, 2 * kk:2 * kk + 2, m * 128:(m + 1) * 128],
                        cur8[:, 2 * kk:2 * kk + 2, :],
                        start=False, stop=False,
                        perf_mode=DR, skip_group_check=True,
                    )
                nc.tensor.matmul(
                    yp[:],
                    u_aug[:, m * 128:(m + 1) * 128],
                    a_aug[:, n * NSL:(n + 1) * NSL],
                    start=False, stop=True, skip_group_check=True,
                )
                y_sb = y_pool.tile([128, NSL], F32, tag="ysb")
                nc.scalar.mul(y_sb[:], yp[:], 1.0 / SW)
                nc.sync.dma_start(
                    y_d[m * 128:(m + 1) * 128, n * NSL:(n + 1) * NSL],
                    y_sb[:],
                )

            # ---- startup: x m=0,1 + W slice 0 + B/A constants ----
            x_transpose_run(0)
            load_x_chunk(2)
            x_transpose_run(1)
            load_x_chunk(3)
            for c in range(CPS):
                w_transpose_run(0, c, 0, 16)
                w_transpose_run(0, c, 16, 16)
            # B: x256 scale + fp8 cast for the fp8 k-range
            nc.vector.tensor_scalar_mul(b_all[:], b_all[:], SW)
            b8_all = const_pool.tile([128, KT8, R], FP8)
            nc.scalar.copy(b8_all[:], b_all[:, KTB:, :])
            # A_aug = [A.T ; 256*bias]
            nc.vector.tensor_scalar_mul(
                a_aug[R:R + 1, :], a_aug[R:R + 1, :], SW
            )
            for o in range(DOUT // 128):
                pt = tpsum_pool.tile([R, 128], BF16, tag="t")
                nc.tensor.transpose(pt[:], a_nat[:, o, :], ident[:])
                nc.vector.tensor_copy(a_aug[0:R, o * 128:(o + 1) * 128], pt[:])
            u_batch(0)

            # ---- n=0: main tiles interleaved with remaining x transposes ----
            for m in range(MT):
                if m == 0:
                    load_w_chunk(1, 0)
                if m % 2 == 1 and (m + 1) // 2 < CPS:
                    load_w_chunk(1, (m + 1) // 2)
                main_tile(0, m)
                w_transpose_run(1, m // 2, (m % 2) * 16, 16)
                # stay one u-batch (two m-tiles) ahead of the main tiles
                if m % 2 == 0 and m + 2 < MT:
                    x_transpose_run(m + 2)
                    if m + 4 < MT:
                        load_x_chunk(m + 4)
                elif m % 2 == 1 and m + 2 < MT:
                    x_transpose_run(m + 2)
                    if m + 4 < MT:
                        load_x_chunk(m + 4)
                    u_batch((m + 1) // 2)

            # ---- main loop over remaining output-feature slices ----
            for n in range(1, NT):
                for m in range(MT):
                    if n + 1 < NT:
                        # chunk c of slice n+1 is consumed at m=2c and 2c+1;
                        # load it one m-iteration ahead (c=0 at m=0).
                        if m == 0:
                            load_w_chunk(n + 1, 0)
                        if m % 2 == 1 and (m + 1) // 2 < CPS:
                            load_w_chunk(n + 1, (m + 1) // 2)
                    main_tile(n, m)
                    if n + 1 < NT:
                        # 16 transposes of slice n+1 after each m's matmuls
                        w_transpose_run(n + 1, m // 2, (m % 2) * 16, 16)

    nc.compile()
    return nc


def _get_nc():
    global _cached
    if _cached is None:
        _cached = _build()
    return _cached


def kernel(x, weight, bias, A, B, _trace=False):
    x = np.ascontiguousarray(np.asarray(x, dtype=np.float32)).reshape(-1, DIN)
    weight = np.ascontiguousarray(np.asarray(weight, dtype=np.float32))
    bias = np.ascontiguousarray(np.asarray(bias, dtype=np.float32))
    A = np.ascontiguousarray(np.asarray(A, dtype=np.float32))
    B = np.ascontiguousarray(np.asarray(B, dtype=np.float32))

    nc = _get_nc()
    in_maps = [
        {
            "x": np.ascontiguousarray(x[c * TOK:(c + 1) * TOK]),
            "weight": weight,
            "bias": bias,
            "A": A,
            "B": B,
        }
        for c in range(N_CORES)
    ]
    res = run_bass_kernel_spmd(
        nc, in_maps, core_ids=list(range(N_CORES)), trace=_trace
    )
    kernel.last_result = res
    y = np.concatenate([res.results[c]["out"] for c in range(N_CORES)], axis=0)
    return y.reshape(4, 2048, DOUT)


kernel.last_result = None
